# revision 5
# baseline (speedup 1.0000x reference)
"""APPNP_Net Trainium2 kernel (8 NeuronCores, SPMD row-sharded).

The reference model is:
    h = relu(x @ W1 + b1); z = h @ W2 + b2; out = log_softmax(z, axis=1)
followed by K=10 APPNP propagation steps with ALPHA=1.0.  Since
z_{t+1} = (1-ALPHA)*agg + ALPHA*h == h, the propagation is the identity
and edge_index never affects the output.  So the kernel is a row-wise
MLP + log_softmax, sharded by nodes across the 8 cores.

Device design:
  - x pre-transposed + pre-tiled on host to [pair, partition, k, row] =
    [13, 128, 4, 1024]: features on SBUF partitions (matmul contracts
    along partitions); each block-pair is ONE 1 MiB DMA with 8 KiB
    contiguous per partition — few triggers (the ~0.6us/trigger Sync
    sequencer cost was a second-order bottleneck) and few descriptors.
  - bf16 matmul operands (x, W1, h, W2), fp32 PSUM/softmax/output.
  - MM1 makes hT [hid, rows] (lhsT = W1 slices); MM2 consumes hT as the
    stationary operand producing z [rows, 50] — the right layout for
    free-dim log_softmax and the output DMA.
  - PE software pipeline: MM2(b-1) is emitted between MM1(b) and
    MM1(b+1), so the PE never waits on the relu producing h(b).
  - ScalarE table discipline: steady loop uses only Relu/Exp (one ACT
    table set); Ln is batched per group (12/12/2 blocks) — the last
    group is tiny so the serial post-PE tail is short.
  - engine balance: ACT relu(mh0)+exp; DVE relu(mh1)+bias-add+reduce+
    broadcast-subtract (stride-0 AP broadcast of the log-sum).
  - log-softmax skips max-subtraction (z is O(1); fp32 exp is safe).
"""

import sys

sys.path.insert(0, "/opt/trn_rl_repo")

import dataclasses
from contextlib import ExitStack

import numpy as np
import ml_dtypes

import concourse.tile as tile
from concourse import bacc, mybir
from concourse.bass_utils import run_bass_kernel_spmd

N_NODES = 100000
F_IN = 512
HID = 256
C = 50
N_CORES = 8
BLOCK = 512
NBLK = 26  # blocks per core
NPAIR = NBLK // 2
ROWS_PER_CORE = NBLK * BLOCK  # 13312; 8*13312 = 106496 >= 100000 (zero-padded)
GROUPS = [(0, 12), (12, 24), (24, 26)]  # log-softmax groups (Ln batching)
KC = F_IN // 128  # 4 contraction chunks for MM1
MH = HID // 128  # 2 hidden chunks
SUB = BLOCK // 128  # 4 row-subtiles per block
GMAX = max(e - s for s, e in GROUPS)

BF16 = mybir.dt.bfloat16
F32 = mybir.dt.float32
np_bf16 = ml_dtypes.bfloat16

_GROUP_OF = {}
for _gi, (_s, _e) in enumerate(GROUPS):
    for _b in range(_s, _e):
        _GROUP_OF[_b] = _gi


def _bcast_cols(ap2d, reps):
    """[P, Q] AP -> [P, Q, reps] AP with a zero-stride inner dim."""
    return dataclasses.replace(ap2d, ap=[ap2d.ap[0], ap2d.ap[1], [0, reps]])


def build_nc():
    nc = bacc.Bacc(
        "TRN2",
        target_bir_lowering=False,
        debug=False,
        num_devices=N_CORES,
    )
    xT = nc.declare_dram_parameter(
        "xT", [NPAIR, 128, KC, 2 * BLOCK], BF16, isOutput=False
    ).ap()
    W1 = nc.declare_dram_parameter("W1", [F_IN, HID], BF16, isOutput=False).ap()
    W2 = nc.declare_dram_parameter("W2", [HID, C], BF16, isOutput=False).ap()
    b1 = nc.declare_dram_parameter("b1", [128, MH], F32, isOutput=False).ap()
    b2b = nc.declare_dram_parameter("b2b", [128, SUB, C], F32, isOutput=False).ap()
    # out[p, q, c] with row = q*128 + p (host transposes back); stores are
    # one DMA per pair with 1.6 KiB contiguous per partition.
    out = nc.declare_dram_parameter(
        "out", [128, NBLK * SUB, C], F32, isOutput=True
    ).ap()

    with tile.TileContext(nc) as tc, ExitStack() as ctx:
        consts = ctx.enter_context(tc.tile_pool(name="consts", bufs=1))
        xpool = ctx.enter_context(tc.tile_pool(name="x", bufs=3))
        hpool = ctx.enter_context(tc.tile_pool(name="h", bufs=2))
        tpool = ctx.enter_context(tc.tile_pool(name="t", bufs=2))
        epool = ctx.enter_context(tc.tile_pool(name="e", bufs=2))
        spool = ctx.enter_context(tc.tile_pool(name="s", bufs=2))
        opool = ctx.enter_context(tc.tile_pool(name="o", bufs=3))
        psum = ctx.enter_context(tc.tile_pool(name="psum", bufs=2, space="PSUM"))

        xts = {}  # pair -> tile [128, KC, 1024] bf16

        def issue_pair(p):
            xt = xpool.tile([128, KC, 2 * BLOCK], BF16, tag="xt", name=f"xt{p}")
            nc.sync.dma_start(out=xt, in_=xT[p])
            xts[p] = xt

        # W1 first (MM1(0) needs it), then the first two x pairs.
        w1sb = []
        for k in range(KC):
            t = consts.tile([128, HID], BF16, tag=f"w1_{k}")
            nc.sync.dma_start(out=t, in_=W1[k * 128 : (k + 1) * 128, :])
            w1sb.append(t)
        issue_pair(0)
        issue_pair(1)
        b1sb = consts.tile([128, MH], F32, tag="b1")
        nc.sync.dma_start(out=b1sb, in_=b1)
        w2sb = []
        for kh in range(MH):
            t = consts.tile([128, C], BF16, tag=f"w2_{kh}")
            nc.sync.dma_start(out=t, in_=W2[kh * 128 : (kh + 1) * 128, :])
            w2sb.append(t)
        b2sb = consts.tile([128, SUB, C], F32, tag="b2")
        nc.sync.dma_start(out=b2sb, in_=b2b)

        hs = {}  # block -> [h0, h1]
        t_gs = {}  # group -> t tile [128, <=GMAX, SUB, C]
        s_gs = {}  # group -> s tile [128, <=GMAX*SUB]
        zos = {}  # pair -> zo tile [128, 2*SUB, C]

        def emit_front(b):
            """x DMA (per pair), MM1, relu for block b."""
            pair, sub = divmod(b, 2)
            if sub == 0 and pair not in xts:
                issue_pair(pair)
            xt = xts[pair]
            hcur = []
            for mh in range(MH):
                ph = psum.tile([128, BLOCK], F32, tag=f"ph{mh}")
                for k in range(KC):
                    nc.tensor.matmul(
                        ph,
                        lhsT=w1sb[k][:, mh * 128 : (mh + 1) * 128],
                        rhs=xt[:, k, sub * BLOCK : (sub + 1) * BLOCK],
                        start=(k == 0),
                        stop=(k == KC - 1),
                    )
                h = hpool.tile([128, BLOCK], BF16, tag=f"h{mh}")
                if mh == 0:
                    # ScalarE: relu(ph + b1)
                    nc.scalar.activation(
                        h,
                        ph,
                        mybir.ActivationFunctionType.Relu,
                        bias=b1sb[:, mh : mh + 1],
                    )
                else:
                    # VectorE: (ph + b1) max 0 — balance the engines
                    nc.vector.tensor_scalar(
                        out=h,
                        in0=ph,
                        scalar1=b1sb[:, mh : mh + 1],
                        scalar2=0.0,
                        op0=mybir.AluOpType.add,
                        op1=mybir.AluOpType.max,
                    )
                hcur.append(h)
            hs[b] = hcur

        def emit_back(b):
            """MM2, bias-add, exp, row-sum for block b."""
            g = _GROUP_OF[b]
            g0, g1 = GROUPS[g]
            j = b - g0
            if j == 0:
                glen = g1 - g0
                t_gs[g] = tpool.tile(
                    [128, glen, SUB, C], F32, tag="t", name=f"t_g{g}"
                )
                s_gs[g] = spool.tile(
                    [128, glen * SUB], F32, tag="s", name=f"s_g{g}"
                )
            t_g, s_g = t_gs[g], s_gs[g]
            hb = hs.pop(b)
            pz = psum.tile([128, SUB, C], F32, tag="pz")
            for rs in range(SUB):
                for kh in range(MH):
                    nc.tensor.matmul(
                        pz[:, rs, :],
                        lhsT=hb[kh][:, rs * 128 : (rs + 1) * 128],
                        rhs=w2sb[kh],
                        start=(kh == 0),
                        stop=(kh == MH - 1),
                    )
            # t = z = pz + b2 (also moves PSUM -> SBUF for the tail)
            nc.vector.tensor_add(t_g[:, j], pz, b2sb)
            e = epool.tile([128, SUB, C], F32, tag="e")
            nc.scalar.activation(e, t_g[:, j], mybir.ActivationFunctionType.Exp)
            nc.vector.reduce_sum(
                out=s_g[:, j * SUB : (j + 1) * SUB],
                in_=e,
                axis=mybir.AxisListType.X,
            )

        def emit_group_tail(g):
            """ls = ln(s); out = z - ls (class-broadcast); store per pair."""
            g0, g1 = GROUPS[g]
            glen = g1 - g0
            t_g, s_g = t_gs.pop(g), s_gs.pop(g)
            ls_g = spool.tile([128, glen * SUB], F32, tag="ls", name=f"ls_g{g}")
            nc.scalar.activation(ls_g, s_g, mybir.ActivationFunctionType.Ln)
            for j in range(glen):
                b = g0 + j
                pair, sub = divmod(b, 2)
                if sub == 0:
                    zos[pair] = opool.tile(
                        [128, 2 * SUB, C], F32, tag="zo", name=f"zo{pair}"
                    )
                zo = zos[pair]
                ls_cols = ls_g[:, j * SUB : (j + 1) * SUB]
                nc.vector.tensor_tensor(
                    out=zo[:, sub * SUB : (sub + 1) * SUB],
                    in0=t_g[:, j],
                    in1=_bcast_cols(ls_cols, C),
                    op=mybir.AluOpType.subtract,
                )
                if sub == 1:
                    q0 = pair * 2 * SUB
                    nc.sync.dma_start(
                        out=out[:, q0 : q0 + 2 * SUB, :], in_=zos.pop(pair)
                    )

        for b in range(NBLK):
            emit_front(b)
            if b >= 1:
                emit_back(b - 1)
                for gi, (g0, g1) in enumerate(GROUPS):
                    if b - 1 == g1 - 1:
                        emit_group_tail(gi)
        emit_back(NBLK - 1)
        emit_group_tail(len(GROUPS) - 1)

    nc.compile()
    return nc


_NC = None


def _get_nc():
    global _NC
    if _NC is None:
        _NC = build_nc()
    return _NC


def make_in_maps(x, W1, b1, W2, b2):
    x = np.asarray(x, dtype=np.float32)
    W1bf = np.asarray(W1, dtype=np.float32).astype(np_bf16)
    W2bf = np.asarray(W2, dtype=np.float32).astype(np_bf16)
    # b1 laid out [p, m]: column m is the per-partition bias of hidden chunk m
    b1t = np.ascontiguousarray(np.asarray(b1, dtype=np.float32).reshape(MH, 128).T)
    b2b = np.ascontiguousarray(
        np.tile(np.asarray(b2, dtype=np.float32), (128, SUB)).reshape(128, SUB, C)
    )

    in_maps = []
    for i in range(N_CORES):
        r0 = i * ROWS_PER_CORE
        r1 = min(r0 + ROWS_PER_CORE, N_NODES)
        shard = np.zeros((ROWS_PER_CORE, F_IN), dtype=np_bf16)
        shard[: r1 - r0] = x[r0:r1].astype(np_bf16)
        # [rows, feat] -> [pair, p, k, r]
        xt = np.ascontiguousarray(
            shard.reshape(NPAIR, 2 * BLOCK, KC, 128).transpose(0, 3, 2, 1)
        )
        in_maps.append({"xT": xt, "W1": W1bf, "W2": W2bf, "b1": b1t, "b2b": b2b})
    return in_maps


def run(x, W1, b1, W2, b2, trace=False, **spmd_kwargs):
    nc = _get_nc()
    in_maps = make_in_maps(x, W1, b1, W2, b2)
    res = run_bass_kernel_spmd(
        nc, in_maps, core_ids=list(range(N_CORES)), trace=trace, **spmd_kwargs
    )
    outs = []
    for i in range(N_CORES):
        o = np.asarray(res.results[i]["out"])  # [128, 104, 50], row = q*128+p
        outs.append(o.transpose(1, 0, 2).reshape(ROWS_PER_CORE, C))
    full = np.concatenate(outs, axis=0)[:N_NODES]
    return np.ascontiguousarray(full.astype(np.float32, copy=False)), res


def kernel(x, edge_index, W1, b1, W2, b2):
    out, _ = run(x, W1, b1, W2, b2, trace=False)
    return out


# revision 6
# speedup vs baseline: 1.0132x; 1.0132x over previous
"""APPNP_Net Trainium2 kernel (8 NeuronCores, SPMD row-sharded).

The reference model is:
    h = relu(x @ W1 + b1); z = h @ W2 + b2; out = log_softmax(z, axis=1)
followed by K=10 APPNP propagation steps with ALPHA=1.0.  Since
z_{t+1} = (1-ALPHA)*agg + ALPHA*h == h, the propagation is the identity
and edge_index never affects the output.  So the kernel is a row-wise
MLP + log_softmax, sharded by nodes across the 8 cores.

Device design:
  - x pre-transposed + pre-tiled on host to [pair, partition, k, row] =
    [13, 128, 4, 1024]: features on SBUF partitions (matmul contracts
    along partitions); each block-pair is ONE 1 MiB DMA with 8 KiB
    contiguous per partition — few triggers (the ~0.6us/trigger Sync
    sequencer cost was a second-order bottleneck) and few descriptors.
  - bf16 matmul operands (x, W1, h, W2), fp32 PSUM/softmax/output.
  - MM1 makes hT [hid, rows] (lhsT = W1 slices); MM2 consumes hT as the
    stationary operand producing z [rows, 50] — the right layout for
    free-dim log_softmax and the output DMA.
  - PE software pipeline: MM2(b-1) is emitted between MM1(b) and
    MM1(b+1), so the PE never waits on the relu producing h(b).
  - ScalarE table discipline: steady loop uses only Relu/Exp (one ACT
    table set); Ln is batched per group (12/12/2 blocks) — the last
    group is tiny so the serial post-PE tail is short.
  - engine balance: ACT relu(mh0)+exp; DVE relu(mh1)+bias-add+reduce+
    broadcast-subtract (stride-0 AP broadcast of the log-sum).
  - log-softmax skips max-subtraction (z is O(1); fp32 exp is safe).
"""

import sys

sys.path.insert(0, "/opt/trn_rl_repo")

import dataclasses
from contextlib import ExitStack

import numpy as np
import ml_dtypes

import concourse.tile as tile
from concourse import bacc, mybir
from concourse.bass_utils import run_bass_kernel_spmd

N_NODES = 100000
F_IN = 512
HID = 256
C = 50
N_CORES = 8
BLOCK = 512
NBLK = 26  # blocks per core
NPAIR = NBLK // 2
ROWS_PER_CORE = NBLK * BLOCK  # 13312; 8*13312 = 106496 >= 100000 (zero-padded)
GROUPS = [(0, 12), (12, 24), (24, 26)]  # log-softmax groups (Ln batching)
KC = F_IN // 128  # 4 contraction chunks for MM1
MH = HID // 128  # 2 hidden chunks
SUB = BLOCK // 128  # 4 row-subtiles per block
GMAX = max(e - s for s, e in GROUPS)

BF16 = mybir.dt.bfloat16
F32 = mybir.dt.float32
np_bf16 = ml_dtypes.bfloat16

_GROUP_OF = {}
for _gi, (_s, _e) in enumerate(GROUPS):
    for _b in range(_s, _e):
        _GROUP_OF[_b] = _gi


def _bcast_cols(ap2d, reps):
    """[P, Q] AP -> [P, Q, reps] AP with a zero-stride inner dim."""
    return dataclasses.replace(ap2d, ap=[ap2d.ap[0], ap2d.ap[1], [0, reps]])


def build_nc():
    nc = bacc.Bacc(
        "TRN2",
        target_bir_lowering=False,
        debug=False,
        num_devices=N_CORES,
    )
    xT = nc.declare_dram_parameter(
        "xT", [NPAIR, 128, KC, 2 * BLOCK], BF16, isOutput=False
    ).ap()
    # W1 packed [p, k, hid]; W2 packed [p, kh, C]; biases packed
    # [p, MH + SUB*C] (b1 columns then b2 broadcast)
    W1p = nc.declare_dram_parameter("W1p", [128, KC, HID], BF16, isOutput=False).ap()
    W2p = nc.declare_dram_parameter("W2p", [128, MH, C], BF16, isOutput=False).ap()
    bc = nc.declare_dram_parameter("bc", [128, MH + SUB * C], F32, isOutput=False).ap()
    # out[p, q, c] with row = q*128 + p (host transposes back); stores are
    # one DMA per pair with 1.6 KiB contiguous per partition.
    out = nc.declare_dram_parameter(
        "out", [128, NBLK * SUB, C], F32, isOutput=True
    ).ap()

    with tile.TileContext(nc) as tc, ExitStack() as ctx:
        consts = ctx.enter_context(tc.tile_pool(name="consts", bufs=1))
        xpool = ctx.enter_context(tc.tile_pool(name="x", bufs=4))
        hpool = ctx.enter_context(tc.tile_pool(name="h", bufs=2))
        tpool = ctx.enter_context(tc.tile_pool(name="t", bufs=2))
        epool = ctx.enter_context(tc.tile_pool(name="e", bufs=2))
        spool = ctx.enter_context(tc.tile_pool(name="s", bufs=2))
        opool = ctx.enter_context(tc.tile_pool(name="o", bufs=3))
        psum = ctx.enter_context(tc.tile_pool(name="psum", bufs=2, space="PSUM"))

        xts = {}  # pair -> tile [128, KC, 1024] bf16

        def issue_pair(p):
            xt = xpool.tile([128, KC, 2 * BLOCK], BF16, tag="xt", name=f"xt{p}")
            nc.sync.dma_start(out=xt, in_=xT[p])
            xts[p] = xt

        # W1 first (MM1(0) needs it), then the first two x pairs.
        w1t = consts.tile([128, KC, HID], BF16, tag="w1")
        nc.sync.dma_start(out=w1t, in_=W1p)
        w1sb = [w1t[:, k, :] for k in range(KC)]
        issue_pair(0)
        issue_pair(1)
        bct = consts.tile([128, MH + SUB * C], F32, tag="bc")
        nc.sync.dma_start(out=bct, in_=bc)
        b1sb = bct[:, :MH]
        b2sb = bct[:, MH:].rearrange("p (s c) -> p s c", s=SUB)
        w2t = consts.tile([128, MH, C], BF16, tag="w2")
        nc.sync.dma_start(out=w2t, in_=W2p)
        w2sb = [w2t[:, kh, :] for kh in range(MH)]

        hs = {}  # block -> [h0, h1]
        t_gs = {}  # group -> t tile [128, <=GMAX, SUB, C]
        s_gs = {}  # group -> s tile [128, <=GMAX*SUB]
        zos = {}  # pair -> zo tile [128, 2*SUB, C]

        def emit_front(b):
            """x DMA (per pair), MM1, relu for block b."""
            pair, sub = divmod(b, 2)
            if sub == 0 and pair not in xts:
                issue_pair(pair)
            xt = xts[pair]
            hcur = []
            for mh in range(MH):
                ph = psum.tile([128, BLOCK], F32, tag=f"ph{mh}")
                for k in range(KC):
                    nc.tensor.matmul(
                        ph,
                        lhsT=w1sb[k][:, mh * 128 : (mh + 1) * 128],
                        rhs=xt[:, k, sub * BLOCK : (sub + 1) * BLOCK],
                        start=(k == 0),
                        stop=(k == KC - 1),
                    )
                h = hpool.tile([128, BLOCK], BF16, tag=f"h{mh}")
                if mh == 0:
                    # ScalarE: relu(ph + b1)
                    nc.scalar.activation(
                        h,
                        ph,
                        mybir.ActivationFunctionType.Relu,
                        bias=b1sb[:, mh : mh + 1],
                    )
                else:
                    # VectorE: (ph + b1) max 0 — balance the engines
                    nc.vector.tensor_scalar(
                        out=h,
                        in0=ph,
                        scalar1=b1sb[:, mh : mh + 1],
                        scalar2=0.0,
                        op0=mybir.AluOpType.add,
                        op1=mybir.AluOpType.max,
                    )
                hcur.append(h)
            hs[b] = hcur

        def emit_back(b):
            """MM2, bias-add, exp, row-sum for block b."""
            g = _GROUP_OF[b]
            g0, g1 = GROUPS[g]
            j = b - g0
            if j == 0:
                glen = g1 - g0
                t_gs[g] = tpool.tile(
                    [128, glen, SUB, C], F32, tag="t", name=f"t_g{g}"
                )
                s_gs[g] = spool.tile(
                    [128, glen * SUB], F32, tag="s", name=f"s_g{g}"
                )
            t_g, s_g = t_gs[g], s_gs[g]
            hb = hs.pop(b)
            pz = psum.tile([128, SUB, C], F32, tag="pz")
            for rs in range(SUB):
                for kh in range(MH):
                    nc.tensor.matmul(
                        pz[:, rs, :],
                        lhsT=hb[kh][:, rs * 128 : (rs + 1) * 128],
                        rhs=w2sb[kh],
                        start=(kh == 0),
                        stop=(kh == MH - 1),
                    )
            # t = z = pz + b2 (also moves PSUM -> SBUF for the tail)
            nc.vector.tensor_add(t_g[:, j], pz, b2sb)
            e = epool.tile([128, SUB, C], F32, tag="e")
            nc.scalar.activation(e, t_g[:, j], mybir.ActivationFunctionType.Exp)
            nc.vector.reduce_sum(
                out=s_g[:, j * SUB : (j + 1) * SUB],
                in_=e,
                axis=mybir.AxisListType.X,
            )

        def emit_group_tail(g):
            """ls = ln(s); out = z - ls (class-broadcast); store per pair."""
            g0, g1 = GROUPS[g]
            glen = g1 - g0
            t_g, s_g = t_gs.pop(g), s_gs.pop(g)
            ls_g = spool.tile([128, glen * SUB], F32, tag="ls", name=f"ls_g{g}")
            nc.scalar.activation(ls_g, s_g, mybir.ActivationFunctionType.Ln)
            for j in range(glen):
                b = g0 + j
                pair, sub = divmod(b, 2)
                if sub == 0:
                    zos[pair] = opool.tile(
                        [128, 2 * SUB, C], F32, tag="zo", name=f"zo{pair}"
                    )
                zo = zos[pair]
                ls_cols = ls_g[:, j * SUB : (j + 1) * SUB]
                nc.vector.tensor_tensor(
                    out=zo[:, sub * SUB : (sub + 1) * SUB],
                    in0=t_g[:, j],
                    in1=_bcast_cols(ls_cols, C),
                    op=mybir.AluOpType.subtract,
                )
                if sub == 1:
                    q0 = pair * 2 * SUB
                    nc.sync.dma_start(
                        out=out[:, q0 : q0 + 2 * SUB, :], in_=zos.pop(pair)
                    )

        for b in range(NBLK):
            emit_front(b)
            if b >= 1:
                emit_back(b - 1)
                for gi, (g0, g1) in enumerate(GROUPS):
                    if b - 1 == g1 - 1:
                        emit_group_tail(gi)
        emit_back(NBLK - 1)
        emit_group_tail(len(GROUPS) - 1)

    nc.compile()
    return nc


_NC = None


def _get_nc():
    global _NC
    if _NC is None:
        _NC = build_nc()
    return _NC


def make_in_maps(x, W1, b1, W2, b2):
    x = np.asarray(x, dtype=np.float32)
    # W1 [512, 256] -> [p, k, hid]; W2 [256, 50] -> [p, kh, C]
    W1p = np.ascontiguousarray(
        np.asarray(W1, dtype=np.float32).astype(np_bf16).reshape(KC, 128, HID).transpose(1, 0, 2)
    )
    W2p = np.ascontiguousarray(
        np.asarray(W2, dtype=np.float32).astype(np_bf16).reshape(MH, 128, C).transpose(1, 0, 2)
    )
    # biases packed [p, MH + SUB*C]: b1 columns then b2 tiled
    b1t = np.asarray(b1, dtype=np.float32).reshape(MH, 128).T
    b2t = np.tile(np.asarray(b2, dtype=np.float32), (128, SUB))
    bc = np.ascontiguousarray(np.concatenate([b1t, b2t], axis=1))

    in_maps = []
    for i in range(N_CORES):
        r0 = i * ROWS_PER_CORE
        r1 = min(r0 + ROWS_PER_CORE, N_NODES)
        shard = np.zeros((ROWS_PER_CORE, F_IN), dtype=np_bf16)
        shard[: r1 - r0] = x[r0:r1].astype(np_bf16)
        # [rows, feat] -> [pair, p, k, r]
        xt = np.ascontiguousarray(
            shard.reshape(NPAIR, 2 * BLOCK, KC, 128).transpose(0, 3, 2, 1)
        )
        in_maps.append({"xT": xt, "W1p": W1p, "W2p": W2p, "bc": bc})
    return in_maps


def run(x, W1, b1, W2, b2, trace=False, **spmd_kwargs):
    nc = _get_nc()
    in_maps = make_in_maps(x, W1, b1, W2, b2)
    res = run_bass_kernel_spmd(
        nc, in_maps, core_ids=list(range(N_CORES)), trace=trace, **spmd_kwargs
    )
    outs = []
    for i in range(N_CORES):
        o = np.asarray(res.results[i]["out"])  # [128, 104, 50], row = q*128+p
        outs.append(o.transpose(1, 0, 2).reshape(ROWS_PER_CORE, C))
    full = np.concatenate(outs, axis=0)[:N_NODES]
    return np.ascontiguousarray(full.astype(np.float32, copy=False)), res


def kernel(x, edge_index, W1, b1, W2, b2):
    out, _ = run(x, W1, b1, W2, b2, trace=False)
    return out


# revision 7
# speedup vs baseline: 1.1821x; 1.1667x over previous
"""APPNP_Net Trainium2 kernel (8 NeuronCores, SPMD row-sharded).

The reference model is:
    h = relu(x @ W1 + b1); z = h @ W2 + b2; out = log_softmax(z, axis=1)
followed by K=10 APPNP propagation steps with ALPHA=1.0.  Since
z_{t+1} = (1-ALPHA)*agg + ALPHA*h == h, the propagation is the identity
and edge_index never affects the output.  So the kernel is a row-wise
MLP + log_softmax, sharded by nodes across the 8 cores.

Device design:
  - x pre-transposed + pre-tiled on host to [pair, partition, k, row] =
    [13, 128, 4, 1024]: features on SBUF partitions (matmul contracts
    along partitions); each block-pair is ONE 1 MiB DMA with 8 KiB
    contiguous per partition — few triggers (the ~0.6us/trigger Sync
    sequencer cost was a second-order bottleneck) and few descriptors.
  - bf16 matmul operands (x, W1, h, W2), fp32 PSUM/softmax/output.
  - MM1 makes hT [hid, rows] (lhsT = W1 slices); MM2 consumes hT as the
    stationary operand producing z [rows, 50] — the right layout for
    free-dim log_softmax and the output DMA.
  - PE software pipeline: MM2(b-1) is emitted between MM1(b) and
    MM1(b+1), so the PE never waits on the relu producing h(b).
  - ScalarE table discipline: steady loop uses only Relu/Exp (one ACT
    table set); Ln is batched per group (12/12/2 blocks) — the last
    group is tiny so the serial post-PE tail is short.
  - engine balance: ACT relu(mh0)+exp; DVE relu(mh1)+bias-add+reduce+
    broadcast-subtract (stride-0 AP broadcast of the log-sum).
  - log-softmax skips max-subtraction (z is O(1); fp32 exp is safe).
"""

import sys

sys.path.insert(0, "/opt/trn_rl_repo")

import dataclasses
from contextlib import ExitStack

import numpy as np
import ml_dtypes

import concourse.tile as tile
from concourse import bacc, mybir
from concourse.bass_utils import run_bass_kernel_spmd

N_NODES = 100000
F_IN = 512
HID = 256
C = 50
N_CORES = 8
BLOCK = 512
NBLK = 26  # blocks per core
NPAIR = NBLK // 2
ROWS_PER_CORE = NBLK * BLOCK  # 13312; 8*13312 = 106496 >= 100000 (zero-padded)
GROUPS = [(0, 12), (12, 24), (24, 26)]  # log-softmax groups (Ln batching)
KC = F_IN // 128  # 4 contraction chunks for MM1
MH = HID // 128  # 2 hidden chunks
SUB = BLOCK // 128  # 4 row-subtiles per block
GMAX = max(e - s for s, e in GROUPS)

BF16 = mybir.dt.bfloat16
F32 = mybir.dt.float32
np_bf16 = ml_dtypes.bfloat16

_GROUP_OF = {}
for _gi, (_s, _e) in enumerate(GROUPS):
    for _b in range(_s, _e):
        _GROUP_OF[_b] = _gi


def _bcast_cols(ap2d, reps):
    """[P, Q] AP -> [P, Q, reps] AP with a zero-stride inner dim."""
    return dataclasses.replace(ap2d, ap=[ap2d.ap[0], ap2d.ap[1], [0, reps]])


def build_nc():
    nc = bacc.Bacc(
        "TRN2",
        target_bir_lowering=False,
        debug=False,
        num_devices=N_CORES,
    )
    xT = nc.declare_dram_parameter(
        "xT", [NPAIR, 128, KC, 2 * BLOCK], BF16, isOutput=False
    ).ap()
    # W1 packed [p, k, hid]; W2 packed [p, kh, C]; biases packed
    # [p, MH + SUB*C] (b1 columns then b2 broadcast)
    W1p = nc.declare_dram_parameter("W1p", [128, KC, HID], BF16, isOutput=False).ap()
    W2p = nc.declare_dram_parameter("W2p", [128, MH, C], BF16, isOutput=False).ap()
    bc = nc.declare_dram_parameter("bc", [128, MH + SUB * C], F32, isOutput=False).ap()
    # out[p, q, c] with row = q*128 + p (host transposes back); stores are
    # one DMA per pair with 1.6 KiB contiguous per partition.
    out = nc.declare_dram_parameter(
        "out", [128, NBLK * SUB, C], F32, isOutput=True
    ).ap()

    with tile.TileContext(nc) as tc, ExitStack() as ctx:
        consts = ctx.enter_context(tc.tile_pool(name="consts", bufs=1))
        xpool = ctx.enter_context(tc.tile_pool(name="x", bufs=4))
        hpool = ctx.enter_context(tc.tile_pool(name="h", bufs=2))
        tpool = ctx.enter_context(tc.tile_pool(name="t", bufs=2))
        epool = ctx.enter_context(tc.tile_pool(name="e", bufs=2))
        spool = ctx.enter_context(tc.tile_pool(name="s", bufs=2))
        opool = ctx.enter_context(tc.tile_pool(name="o", bufs=3))
        psum = ctx.enter_context(tc.tile_pool(name="psum", bufs=2, space="PSUM"))

        xts = {}  # pair -> tile [128, KC, 1024] bf16

        def issue_pair(p, split=False):
            xt = xpool.tile([128, KC, 2 * BLOCK], BF16, tag="xt", name=f"xt{p}")
            if split:
                # 4 parallel DMA queues: cuts the first-tile latency ~4x
                for k in range(KC):
                    nc.sync.dma_start(out=xt[:, k, :], in_=xT[p, :, k, :])
            else:
                nc.sync.dma_start(out=xt, in_=xT[p])
            xts[p] = xt

        # W1 first (MM1(0) needs it), then the first two x pairs.
        w1t = consts.tile([128, KC, HID], BF16, tag="w1")
        nc.sync.dma_start(out=w1t, in_=W1p)
        w1sb = [w1t[:, k, :] for k in range(KC)]
        issue_pair(0, split=True)
        issue_pair(1, split=True)
        bct = consts.tile([128, MH + SUB * C], F32, tag="bc")
        nc.sync.dma_start(out=bct, in_=bc)
        b1sb = bct[:, :MH]
        b2sb = bct[:, MH:].rearrange("p (s c) -> p s c", s=SUB)
        w2t = consts.tile([128, MH, C], BF16, tag="w2")
        nc.sync.dma_start(out=w2t, in_=W2p)
        w2sb = [w2t[:, kh, :] for kh in range(MH)]

        hs = {}  # block -> [h0, h1]
        t_gs = {}  # group -> t tile [128, <=GMAX, SUB, C]
        s_gs = {}  # group -> s tile [128, <=GMAX*SUB]
        zos = {}  # pair -> zo tile [128, 2*SUB, C]

        def emit_front(b):
            """x DMA (per pair), MM1, relu for block b."""
            pair, sub = divmod(b, 2)
            if sub == 0 and pair not in xts:
                issue_pair(pair)
            xt = xts[pair]
            hcur = []
            for mh in range(MH):
                ph = psum.tile([128, BLOCK], F32, tag=f"ph{mh}")
                for k in range(KC):
                    nc.tensor.matmul(
                        ph,
                        lhsT=w1sb[k][:, mh * 128 : (mh + 1) * 128],
                        rhs=xt[:, k, sub * BLOCK : (sub + 1) * BLOCK],
                        start=(k == 0),
                        stop=(k == KC - 1),
                    )
                h = hpool.tile([128, BLOCK], BF16, tag=f"h{mh}")
                if mh == 0:
                    # ScalarE: relu(ph + b1)
                    nc.scalar.activation(
                        h,
                        ph,
                        mybir.ActivationFunctionType.Relu,
                        bias=b1sb[:, mh : mh + 1],
                    )
                else:
                    # VectorE: (ph + b1) max 0 — balance the engines
                    nc.vector.tensor_scalar(
                        out=h,
                        in0=ph,
                        scalar1=b1sb[:, mh : mh + 1],
                        scalar2=0.0,
                        op0=mybir.AluOpType.add,
                        op1=mybir.AluOpType.max,
                    )
                hcur.append(h)
            hs[b] = hcur

        def emit_back(b):
            """MM2, bias-add, exp, row-sum for block b."""
            g = _GROUP_OF[b]
            g0, g1 = GROUPS[g]
            j = b - g0
            if j == 0:
                glen = g1 - g0
                t_gs[g] = tpool.tile(
                    [128, glen, SUB, C], F32, tag="t", name=f"t_g{g}"
                )
                s_gs[g] = spool.tile(
                    [128, glen * SUB], F32, tag="s", name=f"s_g{g}"
                )
            t_g, s_g = t_gs[g], s_gs[g]
            hb = hs.pop(b)
            pz = psum.tile([128, SUB, C], F32, tag="pz")
            for rs in range(SUB):
                for kh in range(MH):
                    nc.tensor.matmul(
                        pz[:, rs, :],
                        lhsT=hb[kh][:, rs * 128 : (rs + 1) * 128],
                        rhs=w2sb[kh],
                        start=(kh == 0),
                        stop=(kh == MH - 1),
                    )
            # t = z = pz + b2 (also moves PSUM -> SBUF for the tail)
            nc.vector.tensor_add(t_g[:, j], pz, b2sb)
            e = epool.tile([128, SUB, C], F32, tag="e")
            nc.scalar.activation(e, t_g[:, j], mybir.ActivationFunctionType.Exp)
            nc.vector.reduce_sum(
                out=s_g[:, j * SUB : (j + 1) * SUB],
                in_=e,
                axis=mybir.AxisListType.X,
            )

        def emit_group_tail(g):
            """ls = ln(s); out = z - ls (class-broadcast); store per pair."""
            g0, g1 = GROUPS[g]
            glen = g1 - g0
            t_g, s_g = t_gs.pop(g), s_gs.pop(g)
            ls_g = spool.tile([128, glen * SUB], F32, tag="ls", name=f"ls_g{g}")
            nc.scalar.activation(ls_g, s_g, mybir.ActivationFunctionType.Ln)
            for j in range(glen):
                b = g0 + j
                pair, sub = divmod(b, 2)
                if sub == 0:
                    zos[pair] = opool.tile(
                        [128, 2 * SUB, C], F32, tag="zo", name=f"zo{pair}"
                    )
                zo = zos[pair]
                ls_cols = ls_g[:, j * SUB : (j + 1) * SUB]
                nc.gpsimd.tensor_tensor(
                    out=zo[:, sub * SUB : (sub + 1) * SUB],
                    in0=t_g[:, j],
                    in1=_bcast_cols(ls_cols, C),
                    op=mybir.AluOpType.subtract,
                )
                if sub == 1:
                    q0 = pair * 2 * SUB
                    nc.sync.dma_start(
                        out=out[:, q0 : q0 + 2 * SUB, :], in_=zos.pop(pair)
                    )

        for b in range(NBLK):
            emit_front(b)
            if b >= 1:
                emit_back(b - 1)
                for gi, (g0, g1) in enumerate(GROUPS):
                    if b - 1 == g1 - 1:
                        emit_group_tail(gi)
        emit_back(NBLK - 1)
        emit_group_tail(len(GROUPS) - 1)

    nc.compile()
    return nc


_NC = None


def _get_nc():
    global _NC
    if _NC is None:
        _NC = build_nc()
    return _NC


def make_in_maps(x, W1, b1, W2, b2):
    x = np.asarray(x, dtype=np.float32)
    # W1 [512, 256] -> [p, k, hid]; W2 [256, 50] -> [p, kh, C]
    W1p = np.ascontiguousarray(
        np.asarray(W1, dtype=np.float32).astype(np_bf16).reshape(KC, 128, HID).transpose(1, 0, 2)
    )
    W2p = np.ascontiguousarray(
        np.asarray(W2, dtype=np.float32).astype(np_bf16).reshape(MH, 128, C).transpose(1, 0, 2)
    )
    # biases packed [p, MH + SUB*C]: b1 columns then b2 tiled
    b1t = np.asarray(b1, dtype=np.float32).reshape(MH, 128).T
    b2t = np.tile(np.asarray(b2, dtype=np.float32), (128, SUB))
    bc = np.ascontiguousarray(np.concatenate([b1t, b2t], axis=1))

    in_maps = []
    for i in range(N_CORES):
        r0 = i * ROWS_PER_CORE
        r1 = min(r0 + ROWS_PER_CORE, N_NODES)
        shard = np.zeros((ROWS_PER_CORE, F_IN), dtype=np_bf16)
        shard[: r1 - r0] = x[r0:r1].astype(np_bf16)
        # [rows, feat] -> [pair, p, k, r]
        xt = np.ascontiguousarray(
            shard.reshape(NPAIR, 2 * BLOCK, KC, 128).transpose(0, 3, 2, 1)
        )
        in_maps.append({"xT": xt, "W1p": W1p, "W2p": W2p, "bc": bc})
    return in_maps


def run(x, W1, b1, W2, b2, trace=False, **spmd_kwargs):
    nc = _get_nc()
    in_maps = make_in_maps(x, W1, b1, W2, b2)
    res = run_bass_kernel_spmd(
        nc, in_maps, core_ids=list(range(N_CORES)), trace=trace, **spmd_kwargs
    )
    outs = []
    for i in range(N_CORES):
        o = np.asarray(res.results[i]["out"])  # [128, 104, 50], row = q*128+p
        outs.append(o.transpose(1, 0, 2).reshape(ROWS_PER_CORE, C))
    full = np.concatenate(outs, axis=0)[:N_NODES]
    return np.ascontiguousarray(full.astype(np.float32, copy=False)), res


def kernel(x, edge_index, W1, b1, W2, b2):
    out, _ = run(x, W1, b1, W2, b2, trace=False)
    return out
